# revision 1
# baseline (speedup 1.0000x reference)
import numpy as np
import jax
import jax.numpy as jnp
from functools import partial

# nn_LinearConvAttention: B=2, DIM=256, H=W=D=48, 4 heads,
# head_dim_qk=32, head_dim_v=64. Sharding: 8 cores = (b, head) pairs,
# fully independent (no communication), per the tensor-parallel head split.
B = 2
DIM = 256
HGRID = 48
NH = 4
DQK = 32
DV = 64
N = HGRID * HGRID * HGRID
EPS = 1e-6


@partial(jax.jit, static_argnums=())
def _shard_compute(x_b, wq_h, wk_h, wv_h, bv_h):
    """One (batch, head) shard on one core.

    x_b   [256, 48, 48, 48] : full-channel input for this batch element
    wq_h  [32, 256], wk_h [32, 256]
    wv_h  [64, 3, 3, 3]     : depthwise taps for this head's 64 v-channels
    bv_h  [64]
    returns out_h [64, N]
    """
    xf = x_b.reshape(DIM, N)

    # 1x1x1 convs == channel matmuls
    q = wq_h @ xf                          # [32, N]
    k = wk_h @ xf                          # [32, N]

    # depthwise 3x3x3 conv, padding 1, on this head's 64 channels
    xh = x_b[:, :, :, :]                   # [256,48,48,48]
    xv = jax.lax.dynamic_slice_in_dim(xh, 0, DIM, 0)  # no-op keep full
    # channels for v of this head are selected on host; here x_b_v passed via wv trick
    # (we pass the pre-sliced 64 channels as the last 64 rows? -> instead host slices)
    return q, k  # placeholder (replaced below)


def _make_shard_fn():
    def f(x_b, x_v, wq_h, wk_h, wv_h, bv_h):
        # x_b  [256, 48,48,48]  full channels (for q/k projections)
        # x_v  [64, 48,48,48]   this head's v-channel slice of x_b
        xf = x_b.reshape(DIM, N)
        q = wq_h @ xf                      # [32, N]
        k = wk_h @ xf                      # [32, N]

        xp = jnp.pad(x_v, ((0, 0), (1, 1), (1, 1), (1, 1)))
        v = jnp.zeros((DV, HGRID, HGRID, HGRID), dtype=x_v.dtype)
        for i in range(3):
            for j in range(3):
                for kk in range(3):
                    w = wv_h[:, i, j, kk][:, None, None, None]
                    v = v + w * jax.lax.slice(
                        xp, (0, i, j, kk), (DV, i + HGRID, j + HGRID, kk + HGRID)
                    )
        v = (v + bv_h[:, None, None, None]).reshape(DV, N)

        q = jax.nn.softmax(q, axis=0)      # over per-head channel dim
        k = jax.nn.softmax(k, axis=1)      # over spatial dim

        kv = k @ v.T                       # [32, 64]
        ksum = k.sum(axis=1)               # [32]
        num = kv.T @ q                     # [64, N]
        Z = ksum @ q                       # [N]
        out = num / (Z[None, :] + EPS)     # [64, N]
        return out

    return jax.jit(f)


_SHARD_FN = None


def kernel(x, Wq, Wk, Wv, bv):
    """Full inputs in, full output out. Shards (b, head) across 8 cores."""
    global _SHARD_FN
    devices = jax.devices()
    assert len(devices) >= 8, devices
    if _SHARD_FN is None:
        _SHARD_FN = _make_shard_fn()

    x = np.asarray(x, dtype=np.float32)
    Wq = np.asarray(Wq, dtype=np.float32)
    Wk = np.asarray(Wk, dtype=np.float32)
    Wv = np.asarray(Wv, dtype=np.float32)
    bv = np.asarray(bv, dtype=np.float32)

    outs = []
    futures = []
    for core in range(8):
        b, h = divmod(core, NH)
        dev = devices[core]
        x_b = jax.device_put(x[b], dev)
        x_v = jax.device_put(x[b, h * DV:(h + 1) * DV], dev)
        wq_h = jax.device_put(Wq[h * DQK:(h + 1) * DQK], dev)
        wk_h = jax.device_put(Wk[h * DQK:(h + 1) * DQK], dev)
        wv_h = jax.device_put(Wv[h * DV:(h + 1) * DV, 0], dev)
        bv_h = jax.device_put(bv[h * DV:(h + 1) * DV], dev)
        futures.append((core, _SHARD_FN(x_b, x_v, wq_h, wk_h, wv_h, bv_h)))

    out = np.empty((B, DIM, N), dtype=np.float32)
    for core, fut in futures:
        b, h = divmod(core, NH)
        out[b, h * DV:(h + 1) * DV] = np.asarray(fut)
    return out.reshape(B, DIM, HGRID, HGRID, HGRID)

